# revision 20
# baseline (speedup 1.0000x reference)
"""Trainium2 Bass kernel for the DeltaNet-style gated linear attention layer.

Full module: qkv+beta projections, RoPE, phi=elu+1, beta-gated rank-1 state
recurrence over T, out-projection, residual, RMSNorm.

Sharding: 8 cores = (batch b, sequence half) pairs; each core owns 1024
contiguous rows of the flattened [B*T, HID] input.  The sequence recurrence is
handled chunkwise (C=128) with the scaling trick; the cross-half state
hand-off is a pairwise AllGather of the final chunk state (cores 2b -> 2b+1),
followed by a per-chunk correction term.
"""
import numpy as np
import ml_dtypes

import concourse.bacc as bacc
import concourse.tile as tile
import concourse.mybir as mybir
from concourse.bass import ts, ds
from concourse.bass_utils import run_bass_kernel_spmd

HID, H, B, T = 1024, 16, 4, 2048
D, C = 64, 128
NCORES = 8
ROWS = B * T // NCORES          # 1024 rows per core
NCH = ROWS // C                 # 8 chunks per core
KT = HID // 128                 # 8 k-tiles over hidden dim
NPAIR = H // 2                  # 8 head pairs
BETA_MIN, BETA_MAX, EPS = 0.8, 0.999, 1e-6
F32 = mybir.dt.float32

AOT = mybir.AluOpType
AF = mybir.ActivationFunctionType


def build(mm_dt=mybir.dt.bfloat16, phases=5, p4sub=4, single=False):
    nc = bacc.Bacc("TRN2", target_bir_lowering=False, debug=False,
                   num_devices=1 if single else NCORES)

    # ---------------- I/O ----------------
    x_s = nc.dram_tensor("x_s", [ROWS, HID], F32, kind="ExternalInput").ap()
    w_all = nc.dram_tensor("w_all", [HID, 3088], mm_dt, kind="ExternalInput").ap()
    wo_t = nc.dram_tensor("wo_t", [HID, HID], mm_dt, kind="ExternalInput").ap()
    cos_i = nc.dram_tensor("cos_i", [ROWS, D], F32, kind="ExternalInput").ap()
    sin_i = nc.dram_tensor("sin_i", [ROWS, D], F32, kind="ExternalInput").ap()
    bob_i = nc.dram_tensor("bob_i", [128, HID], F32, kind="ExternalInput").ap()
    scl_i = nc.dram_tensor("scl_i", [128, HID], F32, kind="ExternalInput").ap()
    bbr_i = nc.dram_tensor("bbr_i", [128, H], F32, kind="ExternalInput").ap()
    ltri_i = nc.dram_tensor("ltri_i", [128, 128], F32, kind="ExternalInput").ap()
    ones_i = nc.dram_tensor("ones_i", [128, 128], F32, kind="ExternalInput").ap()
    id_i = nc.dram_tensor("id_i", [128, 128], mm_dt, kind="ExternalInput").ap()
    one_row_i = nc.dram_tensor("one_row_i", [1, 128], F32, kind="ExternalInput").ap()
    sel_i = nc.dram_tensor("sel_i", [128, 2], F32, kind="ExternalInput").ap()
    out_s = nc.dram_tensor("out_s", [ROWS, HID], F32, kind="ExternalOutput").ap()

    with tile.TileContext(nc) as tc:
        with (
            tc.tile_pool(name="consts", bufs=1) as consts,
            tc.tile_pool(name="dram", bufs=1, space="DRAM") as dram,
            tc.tile_pool(name="arch", bufs=1) as arch,
            tc.tile_pool(name="qkvres", bufs=1) as qkvres,
        ):
            # ------------- constants -------------
            ltri = consts.tile([128, 128], F32)      # [j,i] = 1 if j<=i
            nc.sync.dma_start(ltri[:], ltri_i)
            ones_sq = consts.tile([128, 128], F32)
            nc.sync.dma_start(ones_sq[:], ones_i)
            idm = consts.tile([128, 128], mm_dt)
            nc.sync.dma_start(idm[:], id_i)
            one_row = consts.tile([1, 128], F32)
            nc.sync.dma_start(one_row[:], one_row_i)
            cos_sb = consts.tile([128, NCH, D], F32)
            nc.sync.dma_start(cos_sb[:], cos_i.rearrange("(c p) d -> p c d", p=128))
            sin_sb = consts.tile([128, NCH, D], F32)
            nc.sync.dma_start(sin_sb[:], sin_i.rearrange("(c p) d -> p c d", p=128))
            bbr = consts.tile([128, H], F32)
            nc.sync.dma_start(bbr[:], bbr_i)
            selv = consts.tile([128, 2], F32)
            nc.sync.dma_start(selv[:], sel_i)
            epsb = consts.tile([128, 1], F32)
            nc.vector.memset(epsb[:], EPS)

            # ------------- resident phi(q)/phi(k)/v (bf16) -------------
            phq_all = qkvres.tile([128, NCH, HID], mm_dt)
            phk_all = qkvres.tile([128, NCH, HID], mm_dt)
            v_all = qkvres.tile([128, NCH, HID], mm_dt)
            cfd = dram.tile([NCH, H], F32)
            cc_in = dram.tile([128, NPAIR * 65], F32)
            cc_out = dram.tile([2, 128, NPAIR * 65], F32)

            # ------------- persistent SBUF -------------
            qdi_arch = arch.tile([128, NCH, NPAIR, 128], mm_dt)  # Qtil [d,i] per (c,pair)
            lb_all = arch.tile([128, NCH, H], F32)               # log beta per chunk
            nd_sb = arch.tile([128, NCH, H, 65], F32)            # num/den v1
            cfarch = arch.tile([1, NCH, H], F32)                 # exp(ctot_c)
            attnT = arch.tile([128, KT, ROWS], mm_dt)            # attn^T for out-proj

            # =========== P0/P1: x^T + fused projections ===========
            with (
                tc.tile_pool(name="xload", bufs=3) as xload,
                tc.tile_pool(name="bigx", bufs=1) as bigx,
                tc.tile_pool(name="wpool", bufs=2) as wpool,
                tc.tile_pool(name="ptmp", bufs=3) as ptmp,
                tc.tile_pool(name="ps1", bufs=4, space="PSUM") as ps1,
                tc.tile_pool(name="pst", bufs=2, space="PSUM") as pst,
            ):
                xT = bigx.tile([128, KT, ROWS], mm_dt)
                for i in range(NCH):
                    xi = xload.tile([128, HID], F32, tag="xi")
                    nc.sync.dma_start(xi[:], x_s[ts(i, 128), :])
                    xc = xload.tile([128, HID], mm_dt, tag="xc")
                    nc.gpsimd.tensor_copy(xc[:], xi[:])
                    for k in range(KT):
                        tp = pst.tile([128, 128], mm_dt, tag="tp",
                                      padded_shape=[128, 2048 // mybir.dt.size(mm_dt)])
                        nc.tensor.transpose(tp[:], xc[:, ts(k, 128)], idm[:])
                        nc.scalar.copy(xT[:, k, ts(i, 128)], tp[:])

                # n-blocks: 0..5 = qkv (512 cols each), 6 = beta (16 cols)
                for nb in range(7):
                    ncols = 512 if nb < 6 else 16
                    noff = nb * 512
                    wblk = wpool.tile([128, KT, 512], mm_dt, tag="w")
                    nc.sync.dma_start(
                        wblk[:, :, :ncols],
                        w_all.rearrange("(kt p) n -> p kt n", p=128)[:, :, ds(noff, ncols)],
                    )
                    for i in range(NCH):
                        ps = ps1.tile([128, 512], F32, tag="pp")
                        for k in range(KT):
                            nc.tensor.matmul(
                                ps[:, :ncols], xT[:, k, ts(i, 128)], wblk[:, k, :ncols],
                                start=(k == 0), stop=(k == KT - 1),
                            )
                        if nb < 4:
                            # q (nb 0,1) / k (nb 2,3): rope + phi -> resident
                            half = nb % 2
                            dst = phq_all if nb < 2 else phk_all
                            p3 = ps[:, :].rearrange("p (h two hf) -> p h two hf",
                                                    two=2, hf=32)
                            cosb = cos_sb[:, i, None, :].rearrange(
                                "p o (two hf) -> p o two hf", two=2)
                            sinb = sin_sb[:, i, None, :].rearrange(
                                "p o (two hf) -> p o two hf", two=2)
                            t1 = ptmp.tile([128, 8, 2, 32], F32, tag="t1")
                            nc.vector.tensor_tensor(
                                t1[:], p3, cosb.broadcast_to([128, 8, 2, 32]), op=AOT.mult)
                            t2 = ptmp.tile([128, 8, 2, 32], F32, tag="t2")
                            # shuffled halves: out half 0 <- in half 1, etc.
                            nc.vector.tensor_tensor(
                                t2[:, :, 0, :], p3[:, :, 1, :],
                                sinb[:, :, 0, :].broadcast_to([128, 8, 32]), op=AOT.mult)
                            nc.vector.tensor_tensor(
                                t2[:, :, 1, :], p3[:, :, 0, :],
                                sinb[:, :, 1, :].broadcast_to([128, 8, 32]), op=AOT.mult)
                            s = ptmp.tile([128, 512], F32, tag="s")
                            nc.vector.tensor_add(
                                s[:], t1[:].rearrange("p h two hf -> p (h two hf)"),
                                t2[:].rearrange("p h two hf -> p (h two hf)"))
                            # phi(s) = exp(min(s,0)) + relu(s)
                            mn = ptmp.tile([128, 512], F32, tag="mn")
                            nc.gpsimd.tensor_scalar_min(mn[:], s[:], 0.0)
                            ex = ptmp.tile([128, 512], F32, tag="ex")
                            nc.scalar.activation(ex[:], mn[:], AF.Exp)
                            rl = ptmp.tile([128, 512], F32, tag="rl")
                            nc.scalar.activation(rl[:], s[:], AF.Relu)
                            nc.gpsimd.tensor_add(
                                dst[:, i, ds(half * 512, 512)], ex[:], rl[:])
                        elif nb < 6:
                            half = nb % 2
                            nc.scalar.copy(v_all[:, i, ds(half * 512, 512)],
                                           ps[:, :512])
                        else:
                            # beta: sigmoid(x@Wb.T + bb) -> clip -> log
                            bt = ptmp.tile([128, H], F32, tag="bt")
                            nc.vector.tensor_add(bt[:], ps[:, :H], bbr[:])
                            sg = ptmp.tile([128, H], F32, tag="sg")
                            nc.scalar.activation(sg[:], bt[:], AF.Sigmoid)
                            cl = ptmp.tile([128, H], F32, tag="cl")
                            nc.vector.tensor_scalar(
                                out=cl[:], in0=sg[:], scalar1=BETA_MAX,
                                scalar2=BETA_MIN, op0=AOT.min, op1=AOT.max)
                            nc.scalar.activation(lb_all[:, i, :], cl[:], AF.Ln)

            # =========== P2: chunk recurrence ===========
            with (
                tc.tile_pool(name="ctmp", bufs=2) as ctmp,
                tc.tile_pool(name="spool", bufs=2) as spool,
                tc.tile_pool(name="cc", bufs=1) as ccp,
                tc.tile_pool(name="psat", bufs=2, space="PSUM") as psat,
                tc.tile_pool(name="pstp", bufs=1, space="PSUM") as pstp,
                tc.tile_pool(name="psnd", bufs=3, space="PSUM") as psnd,
                tc.tile_pool(name="psmp", bufs=2, space="PSUM") as psmp,
            ):
                run_ctot = ccp.tile([1, H], F32)
                nc.vector.memset(run_ctot[:], 0.0)
                stil = spool.tile([128, NPAIR, 65], F32, tag="stil")
                nc.vector.memset(stil[:], 0.0)

                # group layout for numden psum tiles: 7 + 7 + 2 heads
                GRP = [(0, 7), (7, 7), (14, 2)]

                for c in range(NCH if phases >= 2 else 0):
                    # --- beta scales ---
                    cum = psat.tile([128, 32], F32, tag="at", name="cum",
                                    padded_shape=[128, 512])
                    nc.tensor.matmul(cum[:, 0:H], ltri[:], lb_all[:, c, :],
                                     start=True, stop=True)
                    nc.tensor.matmul(cum[:, 16:16 + H], ones_sq[:], lb_all[:, c, :],
                                     start=True, stop=True)
                    cums = ctmp.tile([128, 32], F32, tag="cums")
                    nc.scalar.copy(cums[:], cum[:])
                    expP = ctmp.tile([128, H], mm_dt, tag="expP")
                    nc.scalar.activation(expP[:], cums[:, 0:H], AF.Exp)
                    expN = ctmp.tile([128, H], mm_dt, tag="expN")
                    nc.scalar.activation(expN[:], cums[:, 0:H], AF.Exp, scale=-1.0)
                    expT = ctmp.tile([128, H], F32, tag="expT")
                    nc.scalar.activation(expT[:], cums[:, 16:16 + H], AF.Exp)
                    dfc = ctmp.tile([128, H], F32, tag="dfc")
                    nc.vector.tensor_tensor(dfc[:], cums[:, 16:16 + H], cums[:, 0:H],
                                            op=AOT.subtract)
                    expNC = ctmp.tile([128, H], mm_dt, tag="expNC")
                    nc.scalar.activation(expNC[:], dfc[:], AF.Exp)
                    # archive exp(ctot) then update running ctot
                    nc.scalar.activation(cfarch[:, c, :], run_ctot[:], AF.Exp)
                    nc.vector.tensor_add(run_ctot[:], run_ctot[:], cums[0:1, 16:16 + H])
                    # pcb2[p, pp] = P_C(2*pp + (p>=64))
                    expT2 = expT[:].rearrange("p (a two) -> p a two", two=2)
                    pcb2 = ctmp.tile([128, NPAIR], F32, tag="pcb2")
                    nc.vector.tensor_copy(pcb2[0:64, :], expT2[0:64, :, 0])
                    nc.vector.tensor_copy(pcb2[64:128, :], expT2[64:128, :, 1])

                    phq = phq_all[:, c, :]
                    phk = phk_all[:, c, :]
                    vch = v_all[:, c, :]

                    qtil = ctmp.tile([128, H, D], mm_dt, tag="qtil")
                    nc.vector.tensor_tensor(
                        qtil[:], phq.rearrange("p (h d) -> p h d", d=D),
                        expP[:, :, None].broadcast_to([128, H, D]), op=AOT.mult)
                    ktil = ctmp.tile([128, H, D], mm_dt, tag="ktil")
                    nc.vector.tensor_tensor(
                        ktil[:], phk.rearrange("p (h d) -> p h d", d=D),
                        expN[:, :, None].broadcast_to([128, H, D]), op=AOT.mult)
                    kpr = ctmp.tile([128, H, D], mm_dt, tag="kpr")
                    nc.vector.tensor_tensor(
                        kpr[:], phk.rearrange("p (h d) -> p h d", d=D),
                        expNC[:, :, None].broadcast_to([128, H, D]), op=AOT.mult)
                    vt3 = ctmp.tile([128, H, 65], mm_dt, tag="vt3")
                    nc.vector.tensor_copy(
                        vt3[:, :, 0:D], vch.rearrange("p (h d) -> p h d", d=D))
                    nc.vector.memset(vt3[:, :, D:65], 1.0)

                    # --- transposes: qtil/ktil pair blocks -> [d, i] layout ---
                    kdj = ctmp.tile([128, NPAIR, 128], mm_dt, tag="kdj")
                    for p in range(NPAIR):
                        tpq = pstp.tile([128, 128], mm_dt, tag="tpx",
                                        padded_shape=[128, 2048 // mybir.dt.size(mm_dt)])
                        nc.tensor.transpose(
                            tpq[:], qtil[:].rearrange("p h d -> p (h d)")[:, ts(p, 128)],
                            idm[:])
                        nc.vector.tensor_copy(qdi_arch[:, c, p, :], tpq[:])
                        tpk = pstp.tile([128, 128], mm_dt, tag="tpx",
                                        padded_shape=[128, 2048 // mybir.dt.size(mm_dt)])
                        nc.tensor.transpose(
                            tpk[:], ktil[:].rearrange("p h d -> p (h d)")[:, ts(p, 128)],
                            idm[:])
                        nc.scalar.copy(kdj[:, p, :], tpk[:])

                    # f32 state copy in mm dtype for the inter-term matmul
                    stil_mm = ctmp.tile([128, NPAIR, 65], mm_dt, tag="stilmm")
                    nc.scalar.copy(stil_mm[:], stil[:])

                    # --- per-head intra/inter + state delta ---
                    ndt = [psnd.tile([128, n * 65], F32, tag="nd", name=f"nd{gi}",
                                    padded_shape=[128, 512])
                           for gi, (_, n) in enumerate(GRP)]
                    mpt = [psmp.tile([128, 4, 65], F32, tag="mp", name=f"mp{gi}",
                                    padded_shape=[128, 4, 128])
                           for gi in range(2)]
                    for h in range(H):
                        p, par = h // 2, h % 2
                        po = par * 64
                        at = psat.tile([128, 128], F32, tag="at",
                                       padded_shape=[128, 512])
                        nc.tensor.matmul(
                            at[:], kdj[po:po + 64, p, :], qdi_arch[po:po + 64, c, p, :],
                            start=True, stop=True)
                        atm = ctmp.tile([128, 128], mm_dt, tag="atm")
                        nc.vector.tensor_tensor(atm[:], at[:], ltri[:], op=AOT.mult)
                        g = h // 7
                        off = (h % 7) * 65
                        nc.tensor.matmul(ndt[g][:, ds(off, 65)], atm[:], vt3[:, h, :],
                                         start=True, stop=False)
                        nc.tensor.matmul(ndt[g][:, ds(off, 65)],
                                         qdi_arch[po:po + 64, c, p, :],
                                         stil_mm[po:po + 64, p, :],
                                         start=False, stop=True)
                        nc.tensor.matmul(
                            mpt[p // 4][po:po + 64, p % 4, :],
                            kpr[:, h, :], vt3[:, h, :], start=True, stop=True)
                    # numden -> sbuf archive
                    for gi, (h0, nh) in enumerate(GRP):
                        nc.vector.tensor_copy(
                            nd_sb[:, c, ds(h0, nh), :],
                            ndt[gi][:].rearrange("p (h e) -> p h e", e=65))
                    # state update: stil_new = stil * P_C + Mprime
                    snew = spool.tile([128, NPAIR, 65], F32, tag="stil")
                    nc.vector.tensor_tensor(
                        snew[:], stil[:],
                        pcb2[:, :, None].broadcast_to([128, NPAIR, 65]), op=AOT.mult)
                    nc.vector.tensor_tensor(snew[:, 0:4, :], snew[:, 0:4, :],
                                            mpt[0][:], op=AOT.add)
                    nc.vector.tensor_tensor(snew[:, 4:8, :], snew[:, 4:8, :],
                                            mpt[1][:], op=AOT.add)
                    stil = snew

                if phases >= 2:
                    nc.sync.dma_start(cfd[:], cfarch[0])

                # =========== P3: state hand-off ===========
                if phases >= 3:
                    nc.sync.dma_start(cc_in[:], stil[:].rearrange("p a e -> p (a e)"))
                    if single:
                        nc.sync.dma_start(cc_out[0], cc_in[:])
                        nc.sync.dma_start(cc_out[1], cc_in[:])
                    else:
                        nc.gpsimd.collective_compute(
                            "AllGather", AOT.bypass,
                            replica_groups=[[0, 1], [2, 3], [4, 5], [6, 7]],
                            ins=[cc_in[:].opt()],
                            outs=[cc_out[:].opt()],
                        )
                    g0 = ccp.tile([128, NPAIR, 65], F32)
                    nc.sync.dma_start(g0[:], cc_out[0].rearrange("p (a e) -> p a e", e=65))
                    g1 = ccp.tile([128, NPAIR, 65], F32)
                    nc.sync.dma_start(g1[:], cc_out[1].rearrange("p (a e) -> p a e", e=65))
                    shand = ccp.tile([128, NPAIR, 65], F32)
                    nc.vector.tensor_scalar_mul(g1[:], g1[:], selv[:, 1:2])
                    nc.vector.scalar_tensor_tensor(
                        shand[:], g0[:], selv[:, 0:1], g1[:],
                        op0=AOT.mult, op1=AOT.add)

                # =========== P4: corrections + attn finalize ===========
                if phases >= 4:
                    cfb = ccp.tile([128, NCH, H], F32)
                    nc.sync.dma_start(
                        cfb[:], cfd[None, :, :].broadcast_to([128, NCH, H]))
                for c in range(NCH if phases >= 4 else 0):
                    cfb2 = cfb[:, c, :].rearrange("p (a two) -> p a two", two=2)
                    cf2 = ctmp.tile([128, NPAIR], F32, tag="cf2")
                    nc.vector.tensor_copy(cf2[0:64, :], cfb2[0:64, :, 0])
                    nc.vector.tensor_copy(cf2[64:128, :], cfb2[64:128, :, 1])
                    sc = ctmp.tile([128, NPAIR, 65], mm_dt, tag="sc")
                    nc.vector.tensor_tensor(
                        sc[:], shand[:],
                        cf2[:, :, None].broadcast_to([128, NPAIR, 65]), op=AOT.mult)
                    if p4sub < 2:
                        continue
                    # block-diagonal rhs so every correction matmul is full
                    # K=128 (single row-group; avoids concurrent same-bank
                    # PSUM writes from alternating 64-row tile positions)
                    scbd = ctmp.tile([128, NPAIR, 2, 65], mm_dt, tag="scbd")
                    nc.vector.memset(scbd[:], 0.0)
                    nc.vector.tensor_copy(scbd[0:64, :, 0, :], sc[0:64, :, :])
                    nc.vector.tensor_copy(scbd[64:128, :, 1, :], sc[64:128, :, :])
                    GRP4 = [(0, 6), (6, 6), (12, 4)]
                    cot = [psnd.tile([128, (n // 2) * 130], F32, tag="nd",
                                     name=f"co{gi}", padded_shape=[128, 512])
                           for gi, (_, n) in enumerate(GRP4)]
                    for p in range(NPAIR):
                        g = p // 3
                        off = (p % 3) * 130
                        nc.tensor.matmul(cot[g][:, ds(off, 130)],
                                         qdi_arch[:, c, p, :],
                                         scbd[:, p, :, :],
                                         start=True, stop=True)
                    if p4sub < 3:
                        for gi, (_, nh) in enumerate(GRP4):
                            junk = ctmp.tile([128, 7, 65], F32, tag="ndf",
                                             name="junk")
                            nc.vector.tensor_copy(
                                junk[:, 0:nh, :],
                                cot[gi][:].rearrange("p (h e) -> p h e", e=65))
                        continue
                    attn = ctmp.tile([128, HID], mm_dt, tag="attn")
                    for gi, (h0, nh) in enumerate(GRP4):
                        ndf = ctmp.tile([128, 7, 65], F32, tag="ndf")
                        nc.vector.tensor_tensor(
                            ndf[:, 0:nh, :], nd_sb[:, c, ds(h0, nh), :],
                            cot[gi][:].rearrange("p (h e) -> p h e", e=65),
                            op=AOT.add)
                        den = ctmp.tile([128, 7], F32, tag="den")
                        nc.vector.tensor_scalar_add(
                            den[:, 0:nh], ndf[:, 0:nh, 64], EPS)
                        nc.vector.reciprocal(den[:, 0:nh], den[:, 0:nh])
                        nc.vector.tensor_tensor(
                            attn[:].rearrange("p (h d) -> p h d", d=D)[:, ds(h0, nh), :],
                            ndf[:, 0:nh, 0:D],
                            den[:, 0:nh, None].broadcast_to([128, nh, D]),
                            op=AOT.mult)
                    if p4sub < 4:
                        continue
                    for p in range(KT):
                        tpa = pstp.tile([128, 128], mm_dt, tag="tpx",
                                        padded_shape=[128, 2048 // mybir.dt.size(mm_dt)])
                        nc.tensor.transpose(tpa[:], attn[:, ts(p, 128)], idm[:])
                        nc.vector.tensor_copy(attnT[:, p, ts(c, 128)], tpa[:])

            # =========== P5: out-proj + residual + RMSNorm ===========
            with (
                tc.tile_pool(name="wo", bufs=1) as wop,
                tc.tile_pool(name="ftmp", bufs=3) as ftmp,
                tc.tile_pool(name="ps5", bufs=3, space="PSUM") as ps5,
            ):
                bob = wop.tile([128, HID], F32)
                nc.sync.dma_start(bob[:], bob_i)
                sclb = wop.tile([128, HID], F32)
                nc.sync.dma_start(sclb[:], scl_i)
                wo_sb = wop.tile([128, KT, HID], mm_dt)
                nc.sync.dma_start(wo_sb[:], wo_t.rearrange("(kt p) n -> p kt n", p=128))
                if phases < 5:
                    zz = ftmp.tile([128, HID], F32, tag="ysb")
                    nc.vector.memset(zz[:], 0.0)
                    for i in range(NCH):
                        nc.sync.dma_start(out_s[ts(i, 128), :], zz[:])
                for i in range(NCH if phases >= 5 else 0):
                    ysb = ftmp.tile([128, HID], F32, tag="ysb")
                    for nh in range(2):
                        yp = ps5.tile([128, 512], F32, tag="yp")
                        for k in range(KT):
                            nc.tensor.matmul(yp[:], attnT[:, k, ts(i, 128)],
                                             wo_sb[:, k, ds(nh * 512, 512)],
                                             start=(k == 0), stop=(k == KT - 1))
                        nc.vector.tensor_tensor(
                            ysb[:, ds(nh * 512, 512)], yp[:],
                            bob[:, ds(nh * 512, 512)], op=AOT.add)
                    xi = ftmp.tile([128, HID], F32, tag="xi5")
                    nc.sync.dma_start(xi[:], x_s[ts(i, 128), :])
                    hsb = ftmp.tile([128, HID], F32, tag="hsb")
                    nc.vector.tensor_add(hsb[:], ysb[:], xi[:])
                    sq = ftmp.tile([128, HID], F32, tag="sq")
                    ss = ftmp.tile([128, 1], F32, tag="ss")
                    nc.scalar.activation(sq[:], hsb[:], AF.Square, accum_out=ss[:])
                    rms = ftmp.tile([128, 1], F32, tag="rms")
                    nc.scalar.activation(rms[:], ss[:], AF.Sqrt,
                                         bias=epsb[:], scale=1.0 / HID)
                    rinv = ftmp.tile([128, 1], F32, tag="rinv")
                    nc.vector.reciprocal(rinv[:], rms[:])
                    osb = ftmp.tile([128, HID], F32, tag="osb")
                    nc.vector.scalar_tensor_tensor(
                        osb[:], hsb[:], rinv[:], sclb[:],
                        op0=AOT.mult, op1=AOT.mult)
                    nc.sync.dma_start(out_s[ts(i, 128), :], osb[:])

    nc.compile()
    return nc


_CACHED = {}


def _get_nc(mm_dt, phases=5, p4sub=4):
    key = (str(mm_dt), phases, p4sub)
    if key not in _CACHED:
        _CACHED[key] = build(mm_dt, phases, p4sub)
    return _CACHED[key]


def _np_dt(mm_dt):
    return ml_dtypes.bfloat16 if mm_dt == mybir.dt.bfloat16 else np.float32


def prepare_inputs(x, Wq, Wk, Wv, Wb, bb, Wo, bo, scale, mm_dt):
    ndt = _np_dt(mm_dt)
    w_all = np.concatenate([Wq, Wk, Wv, Wb], 0).T.astype(ndt).copy()
    wo_t = Wo.T.astype(ndt).copy()
    inv_freq = 1.0 / (10000.0 ** (np.arange(0, D, 2, dtype=np.float32) / D))
    tt = np.arange(T, dtype=np.float32)
    fr = tt[:, None] * inv_freq[None, :]
    cos_full = np.concatenate([np.cos(fr), np.cos(fr)], 1).astype(np.float32)
    sin_full = np.concatenate([-np.sin(fr), np.sin(fr)], 1).astype(np.float32)
    bob = np.tile(bo[None, :], (128, 1)).astype(np.float32)
    sclb = np.tile(scale[None, :], (128, 1)).astype(np.float32)
    bbr = np.tile(bb[None, :], (128, 1)).astype(np.float32)
    jj, ii = np.meshgrid(np.arange(128), np.arange(128), indexing="ij")
    ltri = (jj <= ii).astype(np.float32)
    ones = np.ones((128, 128), np.float32)
    idm = np.eye(128).astype(ndt)
    one_row = np.ones((1, 128), np.float32)

    xf = x.reshape(B * T, HID).astype(np.float32)
    in_maps = []
    for core in range(NCORES):
        b, half = core // 2, core % 2
        r0 = core * ROWS
        t0 = half * ROWS
        sel = np.zeros((128, 2), np.float32)
        if half == 1:
            sel[:, 0] = 1.0
        in_maps.append({
            "x_s": xf[r0:r0 + ROWS].copy(),
            "w_all": w_all,
            "wo_t": wo_t,
            "cos_i": cos_full[t0:t0 + ROWS].copy(),
            "sin_i": sin_full[t0:t0 + ROWS].copy(),
            "bob_i": bob,
            "scl_i": sclb,
            "bbr_i": bbr,
            "ltri_i": ltri,
            "ones_i": ones,
            "id_i": idm,
            "one_row_i": one_row,
            "sel_i": sel,
        })
    return in_maps


def kernel(x, Wq, Wk, Wv, Wb, bb, Wo, bo, scale,
           mm_dt=mybir.dt.bfloat16, trace=False, phases=5, p4sub=4):
    nc = _get_nc(mm_dt, phases, p4sub)
    in_maps = prepare_inputs(x, Wq, Wk, Wv, Wb, bb, Wo, bo, scale, mm_dt)
    res = run_bass_kernel_spmd(nc, in_maps, core_ids=list(range(NCORES)),
                               trace=trace)
    out = np.zeros((B * T, HID), np.float32)
    for core in range(NCORES):
        out[core * ROWS:(core + 1) * ROWS] = res.results[core]["out_s"]
    out = out.reshape(B, T, HID).astype(x.dtype)
    kernel.last_exec_time_ns = res.exec_time_ns
    return out
